# revision 7
# baseline (speedup 1.0000x reference)
"""MLA (multi-head latent attention) Trainium2 kernel, 8-way sharded.

Strategy (tensor-parallel over heads per the DeepSeek-TP hint, plus
token-parallel stage 1):
  - stage 1 (token-parallel): each core owns 512 tokens (256 from each
    batch). Computes q_a = rmsnorm(x @ Wqa), the rmsnormed compressed
    latent c_kv, and the rotated shared rope key — all in transposed
    (feature x token) layout so no on-device transposes are needed.
    The compressed latent is AllGathered (small); the q side is fully
    projected through Wqb for ALL heads and RoPE'd in stage 1, then
    AllToAll'd so each core receives its 2 heads for all tokens (4x
    fewer collective bytes than AllGathering the q latent).
  - stage 2 (head-TP, 2 heads/core): Wkvb projections, causal
    attention with max-free softmax (scores are bounded ~3, so exp
    never overflows), denominator via on-chip partition-sum matmul.
  - AllToAll attention outputs (each core receives all heads x its own
    tokens), then a token-sharded full-contraction Wo matmul; batch 0's
    Wo overlaps batch 1's A2A.
All heavy matmuls run in bf16 (fp32 is 4 cycles/row on the PE; bf16 is
1 cycle/row). Statistics matmuls (partition sums / broadcasts) run as
float32r (1 cycle/row at free dim >= 256, ~FP22 precision).
"""

import numpy as np
import ml_dtypes

import concourse.bass as bass
import concourse.mybir as mybir
import concourse.tile as tile
from concourse import bacc
from concourse.bass_utils import run_bass_kernel_spmd

BF16 = mybir.dt.bfloat16
F32 = mybir.dt.float32
F32R = mybir.dt.float32r
AF = mybir.ActivationFunctionType

NCORES = 8
B, S, D = 2, 2048, 2048
H = 16
DN, DR, DV = 128, 64, 128
KVR, QR = 512, 1536
T = B * S            # 4096 flattened tokens
TS = T // NCORES     # 512 tokens per core (256 per batch)
HB = TS // B         # 256 tokens per batch per core
HL = H // NCORES     # 2 heads per core
EPS = 1.1920929e-7
BASE = 10000.0
SCALE = 1.0 / float(np.sqrt(DN + DR))

NT = 512             # token free-dim tile
P = 128
KD = D // P          # 16 contraction tiles over model dim
MQ = QR // P         # 12
MC = KVR // P        # 4
NTT = T // NT        # 8 global token n-tiles
CKV = KVR + DR       # 576
KO = H * DV // P     # 16 contraction tiles for Wo
SB = S // NT         # 4 qt tiles per batch
QF = H * (DN + DR)   # 3072 projected q features (all heads)
HPB = QF // NCORES   # 384 q features per head-pair block

_CACHE = {}


# ---------------------------------------------------------------- host side

def _deint_perm():
    return np.concatenate([np.arange(0, DR, 2), np.arange(1, DR, 2)])


def _rope_tables():
    t = np.arange(S, dtype=np.float32)
    inv = 1.0 / (BASE ** (np.arange(0, DR, 2, dtype=np.float32) / DR))
    ang = np.outer(t, inv)
    emb = np.concatenate([ang, ang], axis=-1)          # (S, DR)
    return np.cos(emb), np.sin(emb)


def _shard_tokens(c):
    b0 = np.arange(HB * c, HB * (c + 1))
    return np.concatenate([b0, S + b0])


def host_prep(inputs):
    x = np.asarray(inputs["x"], np.float32).reshape(T, D)
    Wqa = np.asarray(inputs["Wqa"], np.float32)
    gqa = np.asarray(inputs["gqa"], np.float32)
    Wqb = np.asarray(inputs["Wqb"], np.float32)
    Wkva = np.asarray(inputs["Wkva"], np.float32)
    gkva = np.asarray(inputs["gkva"], np.float32)
    Wkvb = np.asarray(inputs["Wkvb"], np.float32)
    Wo = np.asarray(inputs["Wo"], np.float32)

    bf = lambda a: np.ascontiguousarray(a).astype(ml_dtypes.bfloat16)
    perm = _deint_perm()

    wqa_b = bf(Wqa)
    wkva_p = Wkva.copy()
    wkva_p[:, KVR:] = Wkva[:, KVR:][:, perm]
    wkva_b = bf(wkva_p)

    # full Wqb, columns regrouped into 8 head-pair blocks of 384:
    # [h(2j) nope | h(2j+1) nope | h(2j) rope perm | h(2j+1) rope perm]
    blocks = []
    for j in range(NCORES):
        b0 = Wqb[:, (2 * j) * (DN + DR):(2 * j + 1) * (DN + DR)]
        b1 = Wqb[:, (2 * j + 1) * (DN + DR):(2 * j + 2) * (DN + DR)]
        blocks += [b0[:, :DN], b1[:, :DN],
                   b0[:, DN:][:, perm], b1[:, DN:][:, perm]]
    wqb_full = bf(np.concatenate(blocks, axis=1) * gqa[:, None] * SCALE)

    cos, sin = _rope_tables()                          # (S, 64)
    cosT, sinT = cos.T, sin.T                          # (64, S)

    ones_col = np.ones((P, 1), np.float32)
    ones_row = np.ones((1, P), np.float32)
    # rotate-half as a matmul: out = rotm.T @ in  (block-diag over 2 heads)
    HD = DR // 2
    rotm = np.zeros((P, P), np.float32)
    for bb in (0, DR):
        for dout in range(HD):
            rotm[bb + dout + HD, bb + dout] = -1.0
        for dout in range(HD, DR):
            rotm[bb + dout - HD, bb + dout] = 1.0
    rotm_b = bf(rotm)
    wo_b = bf(Wo)

    # diagonal-tile masks, flattened to (128, 4*NT): block jm at cols jm*NT
    qcol = np.arange(NT)
    rr = np.arange(P)
    masks = np.zeros((P, 4 * NT), np.float32)
    for jm in range(4):
        masks[:, jm * NT:(jm + 1) * NT] = (
            qcol[None, :] >= (128 * jm + rr)[:, None])
    masks_b = bf(masks)

    in_maps = []
    for c in range(NCORES):
        toks = _shard_tokens(c)
        xT = bf(x[toks].T)                             # (2048, 512)
        h0, h1 = HL * c, HL * c + 1

        kb0 = Wkvb[:, h0 * (DN + DV):(h0 + 1) * (DN + DV)]
        kb1 = Wkvb[:, h1 * (DN + DV):(h1 + 1) * (DN + DV)]
        wkvbk_c = bf(np.concatenate([kb0[:, :DN], kb1[:, :DN]], axis=1)
                     * gkva[:, None])
        wkvbv_c = bf(np.concatenate([kb0[:, DN:], kb1[:, DN:]], axis=1)
                     * gkva[:, None])

        pos = toks % S
        # cos/sin for this core's tokens, duplicated across both head
        # halves of a rope m-tile (rows 0-63 and 64-127)
        cs = np.ascontiguousarray(cosT[:, pos]).astype(np.float32)
        sn = np.ascontiguousarray(sinT[:, pos]).astype(np.float32)
        cos_s1 = np.concatenate([cs, cs], axis=0)      # (128, TS)
        sin_s1 = np.concatenate([sn, sn], axis=0)

        in_maps.append({
            "xT": xT, "wqa": wqa_b, "wkva": wkva_b, "wqb": wqb_full,
            "wkvbk": wkvbk_c, "wkvbv": wkvbv_c, "wo": wo_b,
            "cos_s1": cos_s1, "sin_s1": sin_s1,
            "masks": masks_b, "ones_col": ones_col, "ones_row": ones_row,
            "rotm": rotm_b,
        })
    return in_maps


# ---------------------------------------------------------------- device IR

def build_nc(do_compile=True):
    nc = bacc.Bacc(
        "TRN2", target_bir_lowering=False, debug=False,
        enable_asserts=True, num_devices=NCORES,
    )
    xT = nc.dram_tensor("xT", [D, TS], BF16, kind="ExternalInput")
    wqa = nc.dram_tensor("wqa", [D, QR], BF16, kind="ExternalInput")
    wkva = nc.dram_tensor("wkva", [D, CKV], BF16, kind="ExternalInput")
    wqb = nc.dram_tensor("wqb", [QR, QF], BF16, kind="ExternalInput")
    wkvbk = nc.dram_tensor("wkvbk", [KVR, HL * DN], BF16, kind="ExternalInput")
    wkvbv = nc.dram_tensor("wkvbv", [KVR, HL * DV], BF16, kind="ExternalInput")
    wo = nc.dram_tensor("wo", [H * DV, D], BF16, kind="ExternalInput")
    cos_s1 = nc.dram_tensor("cos_s1", [P, TS], F32, kind="ExternalInput")
    sin_s1 = nc.dram_tensor("sin_s1", [P, TS], F32, kind="ExternalInput")
    masks = nc.dram_tensor("masks", [P, 4 * NT], BF16, kind="ExternalInput")
    ones_col = nc.dram_tensor("ones_col", [P, 1], F32R, kind="ExternalInput")
    ones_row = nc.dram_tensor("ones_row", [1, P], F32R, kind="ExternalInput")
    rotm = nc.dram_tensor("rotm", [P, P], BF16, kind="ExternalInput")
    out = nc.dram_tensor("out", [TS, D], F32, kind="ExternalOutput")

    RG = [list(range(NCORES))]

    with tile.TileContext(nc) as tc:
        with (
            tc.tile_pool(name="const", bufs=1) as cpool,
            tc.tile_pool(name="dram", bufs=1, space="DRAM") as dram,
        ):
            onc = cpool.tile([P, 1], F32R, name="onc")
            onr = cpool.tile([1, P], F32R, name="onr")
            nc.sync.dma_start(onc[:], ones_col[:])
            nc.sync.dma_start(onr[:], ones_row[:])
            mask_sb = cpool.tile([P, 4 * NT], BF16, name="mask_sb")
            nc.sync.dma_start(mask_sb[:], masks[:])
            eps_sb = cpool.tile([1, 1], F32, name="eps_sb")
            nc.vector.memset(eps_sb[:], EPS)
            rotm_sb = cpool.tile([P, P], BF16, name="rotm_sb")
            nc.sync.dma_start(rotm_sb[:], rotm[:])

            cc_kv_in = dram.tile([CKV, TS], BF16, name="cc_kv_in")
            cc_kv_out = dram.tile([NCORES * CKV, TS], BF16,
                                  addr_space="Shared", name="cc_kv_out")
            cc_q_in = dram.tile([QF, TS], BF16, name="cc_q_in")
            cc_q_out = dram.tile([QF, TS], BF16, name="cc_q_out")
            cc_ao_in = [dram.tile([NCORES * HL * DV, HB], BF16,
                                  name=f"cc_ao_in{b}") for b in range(B)]
            cc_ao_out = [dram.tile([NCORES * HL * DV, HB], BF16,
                                   name=f"cc_ao_out{b}") for b in range(B)]

            # =================== stage 1 =================================
            with (
                tc.tile_pool(name="s1sb", bufs=1) as s1,
                tc.tile_pool(name="s1ps", bufs=1, space="PSUM") as ps1,
            ):
                xT_sb = s1.tile([P, KD * TS], BF16, name="xT_sb")
                wkva_sb = s1.tile([P, KD * CKV], BF16, name="wkva_sb")
                wqa_sb = s1.tile([P, KD * QR], BF16, name="wqa_sb")
                for k in range(KD):
                    nc.sync.dma_start(xT_sb[:, k * TS:(k + 1) * TS],
                                      xT[k * P:(k + 1) * P, :])
                    nc.sync.dma_start(wkva_sb[:, k * CKV:(k + 1) * CKV],
                                      wkva[k * P:(k + 1) * P, :])
                    nc.sync.dma_start(wqa_sb[:, k * QR:(k + 1) * QR],
                                      wqa[k * P:(k + 1) * P, :])
                cos1_sb = s1.tile([P, TS], F32, name="cos1_sb")
                sin1_sb = s1.tile([P, TS], F32, name="sin1_sb")
                nc.sync.dma_start(cos1_sb[:], cos_s1[:])
                nc.sync.dma_start(sin1_sb[:], sin_s1[:])

                # ---- kv side first (its AllGather overlaps the q side)
                c_ps = []
                for m in range(MC):
                    ps = ps1.tile([P, TS], F32, tag="s1mm", bufs=4,
                                  name=f"ckv_ps{m}")
                    for k in range(KD):
                        nc.tensor.matmul(
                            ps[:],
                            wkva_sb[:, k * CKV + m * P:k * CKV + (m + 1) * P],
                            xT_sb[:, k * TS:(k + 1) * TS],
                            start=(k == 0), stop=(k == KD - 1))
                    c_ps.append(ps)
                kr_ps = ps1.tile([DR, TS], F32, tag="s1kr", bufs=2,
                                 name="kr_ps")
                for k in range(KD):
                    nc.tensor.matmul(
                        kr_ps[:],
                        wkva_sb[:, k * CKV + KVR:k * CKV + KVR + DR],
                        xT_sb[:, k * TS:(k + 1) * TS],
                        start=(k == 0), stop=(k == KD - 1))

                # rotate-half via PE: krot = rotm.T @ kr
                kraw = s1.tile([DR, TS], BF16, name="kraw")
                nc.scalar.copy(kraw[:], kr_ps[:])
                krot_ps = ps1.tile([DR, TS], F32, tag="s1kr", bufs=2,
                                   name="krot_ps")
                nc.tensor.matmul(krot_ps[:], rotm_sb[0:DR, 0:DR], kraw[:],
                                 start=True, stop=True)
                t1 = s1.tile([DR, TS], F32, name="t1")
                t2 = s1.tile([DR, TS], F32, name="t2")
                nc.vector.tensor_mul(t1[:], kr_ps[:], cos1_sb[0:DR, :])
                nc.vector.tensor_mul(t2[:], krot_ps[:], sin1_sb[0:DR, :])
                kro_sb = s1.tile([DR, TS], BF16, name="kro_sb")
                nc.vector.tensor_add(kro_sb[:], t1[:], t2[:])
                nc.sync.dma_start(cc_kv_in[KVR:CKV, :], kro_sb[:])

                ss_ps = ps1.tile([1, TS], F32, tag="s1row", bufs=1,
                                 name="ss_kv")
                for m in range(MC):
                    sq = s1.tile([P, TS], F32R, tag="sq", bufs=2,
                                 name=f"sqkv{m}")
                    nc.scalar.square(sq[:], c_ps[m][:])
                    nc.tensor.matmul(ss_ps[:], onc[:], sq[:],
                                     start=(m == 0), stop=(m == MC - 1))
                srow = s1.tile([1, TS], F32, tag="srow", bufs=2,
                               name="srow_kv")
                nc.scalar.activation(srow[:], ss_ps[:], AF.Sqrt,
                                     bias=eps_sb[:], scale=1.0 / KVR)
                rrow = s1.tile([1, TS], F32R, tag="rrow", bufs=2,
                               name="rrow_kv")
                with nc.allow_low_precision(reason="f32r feeds f32r matmul"):
                    nc.vector.reciprocal(rrow[:], srow[:])
                bc_ps = ps1.tile([P, TS], F32, tag="s1bc", bufs=1,
                                 name="bc_kv")
                nc.tensor.matmul(bc_ps[:], onr[:], rrow[:],
                                 start=True, stop=True)
                bc_sb = s1.tile([P, TS], F32, tag="bcs", bufs=2,
                                name="bcs_kv")
                nc.scalar.copy(bc_sb[:], bc_ps[:])
                for m in range(MC):
                    cn = s1.tile([P, TS], BF16, tag="cn", bufs=2,
                                 name=f"cn{m}")
                    nc.vector.tensor_mul(cn[:], c_ps[m][:], bc_sb[:])
                    nc.sync.dma_start(cc_kv_in[m * P:(m + 1) * P, :], cn[:])

                nc.gpsimd.collective_compute(
                    "AllGather", mybir.AluOpType.bypass, replica_groups=RG,
                    ins=[cc_kv_in.opt()], outs=[cc_kv_out.opt()])

                # ---- q side: q_a = rmsnorm(x @ Wqa) kept in SBUF as bf16
                ssq_ps = ps1.tile([1, TS], F32, tag="s1row", bufs=1,
                                  name="ss_q")
                qa_raw = []
                for m in range(MQ):
                    ps = ps1.tile([P, TS], F32, tag="s1mm", bufs=4,
                                  name=f"qa_ps{m}")
                    for k in range(KD):
                        nc.tensor.matmul(
                            ps[:],
                            wqa_sb[:, k * QR + m * P:k * QR + (m + 1) * P],
                            xT_sb[:, k * TS:(k + 1) * TS],
                            start=(k == 0), stop=(k == KD - 1))
                    raw = s1.tile([P, TS], BF16, tag=f"qraw{m}", bufs=1,
                                  name=f"qraw{m}")
                    nc.vector.tensor_copy(raw[:], ps[:])
                    qa_raw.append(raw)
                    sq = s1.tile([P, TS], F32R, tag="sq", bufs=2,
                                 name=f"sqq{m}")
                    nc.scalar.square(sq[:], ps[:])
                    nc.tensor.matmul(ssq_ps[:], onc[:], sq[:],
                                     start=(m == 0), stop=(m == MQ - 1))
                srow_q = s1.tile([1, TS], F32, tag="srow", bufs=2,
                                 name="srow_q")
                nc.scalar.activation(srow_q[:], ssq_ps[:], AF.Sqrt,
                                     bias=eps_sb[:], scale=1.0 / QR)
                rrow_q = s1.tile([1, TS], F32R, tag="rrow", bufs=2,
                                 name="rrow_q")
                with nc.allow_low_precision(reason="f32r feeds f32r matmul"):
                    nc.vector.reciprocal(rrow_q[:], srow_q[:])
                bcq_ps = ps1.tile([P, TS], F32, tag="s1bc", bufs=1,
                                  name="bc_q")
                nc.tensor.matmul(bcq_ps[:], onr[:], rrow_q[:],
                                 start=True, stop=True)
                bcq_sb = s1.tile([P, TS], F32, tag="bcs", bufs=2,
                                 name="bcs_q")
                nc.scalar.copy(bcq_sb[:], bcq_ps[:])
                qn = []
                for m in range(MQ):
                    q = s1.tile([P, TS], BF16, tag=f"qn{m}", bufs=1,
                                name=f"qn{m}")
                    nc.vector.tensor_mul(q[:], qa_raw[m][:], bcq_sb[:])
                    qn.append(q)

                # ---- full q projection for ALL heads + rope, then A2A.
                # wqb streamed in 8 chunks of one head-pair block each
                # (k-split layout [P, MQ * 384] per chunk, double-buffered).
                CW = HPB          # 384 columns per chunk
                for j in range(NCORES):
                    wq = s1.tile([P, MQ * CW], BF16, tag="wqbc", bufs=2,
                                 name=f"wqbc{j}")
                    for k in range(MQ):
                        nc.sync.dma_start(
                            wq[:, k * CW:(k + 1) * CW],
                            wqb[k * P:(k + 1) * P,
                                j * CW:(j + 1) * CW])
                    if True:
                        for m in range(3):
                            ps = ps1.tile([P, TS], F32, tag="s1mm", bufs=4,
                                          name="qf_ps")
                            for k in range(MQ):
                                nc.tensor.matmul(
                                    ps[:],
                                    wq[:, k * CW + m * P:
                                       k * CW + (m + 1) * P],
                                    qn[k][:],
                                    start=(k == 0), stop=(k == MQ - 1))
                            if m < 2:
                                qf = s1.tile([P, TS], BF16, tag="qf", bufs=3,
                                             name="qf")
                                nc.scalar.copy(qf[:], ps[:])
                                nc.sync.dma_start(
                                    cc_q_in[j * HPB + m * P:
                                            j * HPB + (m + 1) * P, :], qf[:])
                            else:
                                qraw = s1.tile([P, TS], BF16, tag="qrr",
                                               bufs=2, name="qrr")
                                nc.scalar.copy(qraw[:], ps[:])
                                rps = ps1.tile([P, TS], F32, tag="s1kr",
                                               bufs=2, name="rps")
                                nc.tensor.matmul(rps[:], rotm_sb[:], qraw[:],
                                                 start=True, stop=True)
                                u1 = s1.tile([P, TS], F32, tag="u1", bufs=2,
                                             name="u1")
                                u2 = s1.tile([P, TS], F32, tag="u2", bufs=2,
                                             name="u2")
                                nc.vector.tensor_mul(u1[:], ps[:], cos1_sb[:])
                                nc.vector.tensor_mul(u2[:], rps[:], sin1_sb[:])
                                qro = s1.tile([P, TS], BF16, tag="qf", bufs=3,
                                              name="qro")
                                nc.vector.tensor_add(qro[:], u1[:], u2[:])
                                nc.sync.dma_start(
                                    cc_q_in[j * HPB + 2 * P:
                                            j * HPB + 3 * P, :], qro[:])

                nc.gpsimd.collective_compute(
                    "AllToAll", mybir.AluOpType.bypass, replica_groups=RG,
                    ins=[cc_q_in.opt()], outs=[cc_q_out.opt()])

            # =================== stage 2 =================================
            ckv_g = cc_kv_out.rearrange("(r p) t -> r p t", r=NCORES)
            q_g = cc_q_out.rearrange("(r f) t -> r f t", r=NCORES)

            with (
                tc.tile_pool(name="s2sb", bufs=1) as s2,
                tc.tile_pool(name="attnsb", bufs=1) as sA,
                tc.tile_pool(name="s2ps", bufs=2, space="PSUM") as ps2,
                tc.tile_pool(name="attnps", bufs=1, space="PSUM") as psA,
            ):
                kn_sb = s2.tile([P, HL * T], BF16, name="kn_sb")
                qn_sb = s2.tile([P, HL * T], BF16, name="qn_sb")
                qr_sb = s2.tile([P, T], BF16, name="qr_sb")
                # shared rotated rope key, duplicated in both partition
                # halves so lhsT/rhs base partitions match per head
                kro2_sb = s2.tile([P, T], BF16, name="kro2_sb")
                v_tiles = [s2.tile([P, HL * DV], BF16, tag=f"v{tt}", bufs=1,
                                   name=f"v{tt}") for tt in range(T // P)]

                with tc.tile_pool(name="projsb", bufs=1) as pj:
                    wkvbk_sb = pj.tile([P, MC * HL * DN], BF16,
                                       name="wkvbk_sb")
                    wkvbv_sb = pj.tile([P, MC * HL * DV], BF16,
                                       name="wkvbv_sb")
                    for k in range(MC):
                        nc.sync.dma_start(
                            wkvbk_sb[:, k * HL * DN:(k + 1) * HL * DN],
                            wkvbk[k * P:(k + 1) * P, :])
                        nc.sync.dma_start(
                            wkvbv_sb[:, k * HL * DV:(k + 1) * HL * DV],
                            wkvbv[k * P:(k + 1) * P, :])

                    # assemble gathered c_kv / rope key into natural order
                    c_sb = pj.tile([P, MC * T], BF16, name="c_sb")
                    c_v = c_sb.rearrange("p (k b t) -> p k b t", k=MC, b=B)
                    kro_v0 = kro2_sb[0:DR, :].rearrange("p (b t) -> p b t",
                                                        b=B)
                    kro_v1 = kro2_sb[DR:P, :].rearrange("p (b t) -> p b t",
                                                        b=B)
                    for r in range(NCORES):
                        src = ckv_g[r].rearrange("p (b t) -> p b t", b=B)
                        for k in range(MC):
                            nc.sync.dma_start(
                                c_v[:, k, :, HB * r:HB * (r + 1)],
                                src[k * P:(k + 1) * P, :, :])
                        nc.sync.dma_start(
                            kro_v0[:, :, HB * r:HB * (r + 1)],
                            src[KVR:CKV, :, :])
                        nc.sync.dma_start(
                            kro_v1[:, :, HB * r:HB * (r + 1)],
                            src[KVR:CKV, :, :])

                    # kT projection
                    for h in range(HL):
                        for n in range(NTT):
                            ps = ps2.tile([P, NT], F32, tag="proj",
                                          name="kn_ps")
                            for k in range(MC):
                                nc.tensor.matmul(
                                    ps[:],
                                    wkvbk_sb[:, k * HL * DN + h * DN:
                                             k * HL * DN + (h + 1) * DN],
                                    c_sb[:, k * T + n * NT:
                                         k * T + (n + 1) * NT],
                                    start=(k == 0), stop=(k == MC - 1))
                            nc.scalar.copy(
                                kn_sb[:, h * T + n * NT:h * T + (n + 1) * NT],
                                ps[:])

                    # v projection (natural layout)
                    for tt in range(T // P):
                        ps = ps2.tile([P, HL * DV], F32, tag="proj",
                                      name="v_ps")
                        for k in range(MC):
                            nc.tensor.matmul(
                                ps[:],
                                c_sb[:, k * T + tt * P:k * T + (tt + 1) * P],
                                wkvbv_sb[:, k * HL * DV:(k + 1) * HL * DV],
                                start=(k == 0), stop=(k == MC - 1))
                        nc.scalar.copy(v_tiles[tt][:], ps[:])

                    # assemble this core's 2 heads' q from the A2A output
                    qn_v = [qn_sb[:, h * T:(h + 1) * T]
                            .rearrange("p (b t) -> p b t", b=B)
                            for h in range(HL)]
                    qr_v = qr_sb.rearrange("p (b t) -> p b t", b=B)
                    for r in range(NCORES):
                        src = q_g[r].rearrange("f (b t) -> f b t", b=B)
                        for h in range(HL):
                            nc.sync.dma_start(
                                qn_v[h][:, :, HB * r:HB * (r + 1)],
                                src[h * P:(h + 1) * P, :, :])
                        nc.sync.dma_start(
                            qr_v[:, :, HB * r:HB * (r + 1)],
                            src[2 * P:3 * P, :, :])

                # ---- Wo weights: load early, overlaps attention ----
                wp = tc.alloc_tile_pool(name="wosb", bufs=1)
                wo_sb = wp.tile([P, KO * D], BF16, name="wo_sb")
                for k in range(KO):
                    nc.sync.dma_start(wo_sb[:, k * D:(k + 1) * D],
                                      wo[k * P:(k + 1) * P, :])

                def wo_block(b):
                    for mb in range(HB // P):            # 2 m-tiles per batch
                        aog = []
                        for k in range(KO):
                            ag = wp.tile([P, P], BF16, tag="aog",
                                         bufs=KO + 2, name=f"aog{k}")
                            nc.sync.dma_start(
                                ag[:],
                                cc_ao_out[b][k * P:(k + 1) * P,
                                             mb * P:(mb + 1) * P])
                            aog.append(ag)
                        for n in range(D // NT):
                            ps = ps2.tile([P, NT], F32, tag="proj",
                                          name="wo_ps")
                            for k in range(KO):
                                nc.tensor.matmul(
                                    ps[:], aog[k][:],
                                    wo_sb[:, k * D + n * NT:
                                          k * D + (n + 1) * NT],
                                    start=(k == 0), stop=(k == KO - 1))
                            ob = wp.tile([P, NT], F32, tag="ob", bufs=3,
                                         name="ob")
                            nc.scalar.copy(ob[:], ps[:])
                            nc.sync.dma_start(
                                out[(b * 2 + mb) * P:
                                    (b * 2 + mb + 1) * P,
                                    n * NT:(n + 1) * NT], ob[:])

                # ---- attention ----
                ao_sb = [sA.tile([DV, T], BF16, name=f"ao_sb{h}")
                         for h in range(HL)]
                for b in range(B):
                    for h in range(HL):
                        for qti in range(SB):
                            qs = b * S + qti * NT
                            aop = psA.tile([DV, NT], F32, tag="ao", bufs=2,
                                           name="aop")
                            dn = sA.tile([P, NT], F32R, tag="dn", bufs=2,
                                         name="dn")
                            nk = 4 * qti + 4
                            for kti in range(nk):
                                ks = b * S + kti * P
                                scp = psA.tile([P, NT], F32, tag="sc",
                                               bufs=2, name="scp")
                                nc.tensor.matmul(
                                    scp[:],
                                    kn_sb[:, h * T + ks:h * T + ks + P],
                                    qn_sb[:, h * T + qs:h * T + qs + NT],
                                    start=True, stop=False)
                                nc.tensor.matmul(
                                    scp[:],
                                    kro2_sb[h * DR:(h + 1) * DR, ks:ks + P],
                                    qr_sb[h * DR:(h + 1) * DR, qs:qs + NT],
                                    start=False, stop=True)
                                et = sA.tile([P, NT], BF16, tag="et", bufs=3,
                                             name="et")
                                nc.scalar.activation(et[:], scp[:], AF.Exp)
                                if kti >= 4 * qti:
                                    jm = kti % 4
                                    nc.vector.tensor_mul(
                                        et[:], et[:],
                                        mask_sb[:, jm * NT:(jm + 1) * NT])
                                if kti == 0:
                                    nc.vector.tensor_copy(dn[:], et[:])
                                else:
                                    nc.vector.tensor_add(dn[:], dn[:], et[:])
                                tt = (b * S) // P + kti
                                nc.tensor.matmul(
                                    aop[:],
                                    v_tiles[tt][:, h * DV:(h + 1) * DV],
                                    et[:],
                                    start=(kti == 0), stop=(kti == nk - 1))
                            dps = psA.tile([1, NT], F32, tag="drow", bufs=1,
                                           name="dps")
                            nc.tensor.matmul(dps[:], onc[:], dn[:],
                                             start=True, stop=True)
                            rec = sA.tile([1, NT], F32R, tag="rec", bufs=2,
                                          name="rec")
                            with nc.allow_low_precision(
                                    reason="f32r feeds f32r matmul"):
                                nc.vector.reciprocal(rec[:], dps[:])
                            bcp = psA.tile([P, NT], F32, tag="bc", bufs=1,
                                           name="bcp")
                            nc.tensor.matmul(bcp[:], onr[:], rec[:],
                                             start=True, stop=True)
                            bcs = sA.tile([P, NT], F32, tag="bcs2", bufs=2,
                                          name="bcs")
                            nc.scalar.copy(bcs[:], bcp[:])
                            nc.vector.tensor_mul(
                                ao_sb[h][:, qs:qs + NT], aop[:], bcs[:])
                    # AllToAll this batch: shard j = (my heads, core j toks)
                    for j in range(NCORES):
                        for h in range(HL):
                            nc.sync.dma_start(
                                cc_ao_in[b][(j * HL + h) * DV:
                                            (j * HL + h + 1) * DV, :],
                                ao_sb[h][:, b * S + HB * j:
                                         b * S + HB * (j + 1)])
                    nc.gpsimd.collective_compute(
                        "AllToAll", mybir.AluOpType.bypass, replica_groups=RG,
                        ins=[cc_ao_in[b].opt()], outs=[cc_ao_out[b].opt()])
                    if b == B - 1:
                        wo_block(0)

                # ---- Wo for batch 1 (batch 0 ran during batch 1's A2A)
                wo_block(1)
                wp.release()

    if do_compile:
        nc.compile()
    return nc


# ---------------------------------------------------------------- entry

def _get_nc():
    if "nc" not in _CACHE:
        _CACHE["nc"] = build_nc()
    return _CACHE["nc"]


def kernel(**inputs):
    nc = _get_nc()
    in_maps = host_prep(inputs)
    res = run_bass_kernel_spmd(nc, in_maps, core_ids=list(range(NCORES)))
    outg = np.zeros((T, D), np.float32)
    for c in range(NCORES):
        outg[_shard_tokens(c)] = res.results[c]["out"]
    return outg.reshape(B, S, D)


# revision 15
# speedup vs baseline: 1.1999x; 1.1999x over previous
"""MLA (multi-head latent attention) Trainium2 kernel, 8-way sharded.

Strategy (tensor-parallel over heads per the DeepSeek-TP hint, plus
token-parallel stage 1):
  - stage 1 (token-parallel): each core owns 512 tokens (256 from each
    batch). Computes the rmsnormed compressed latent + rotated shared
    rope key (AllGathered, small), and the q path: x @ Wqa -> projected
    through the full Wqb for ALL heads on the *unnormalized* latent
    (the per-token rms scale commutes with the feature contraction and
    is applied to the projection outputs), RoPE'd, then AllToAll'd in
    two pieces (nope features first so the collective overlaps the
    rope-side compute) so each core receives its 2 heads for all
    tokens.
  - stage 2 (head-TP, 2 heads/core): Wkvb projections (overlap the q
    collectives), causal attention with max-free softmax (scores are
    bounded ~3, so exp never overflows; exp batched over two key
    blocks per activation), denominator via on-chip partition-sum
    matmul.
  - AllToAll attention outputs per batch (each core receives all heads
    x its own tokens), then a token-sharded full-contraction Wo
    matmul; batch 0's Wo overlaps batch 1's A2A.
All heavy matmuls run in bf16 (fp32 is 4 cycles/row on the PE; bf16 is
1 cycle/row). Statistics matmuls (partition sums / broadcasts) run as
float32r (1 cycle/row at free dim >= 256, ~FP22 precision).
"""

import numpy as np
import ml_dtypes

import concourse.bass as bass
import concourse.mybir as mybir
import concourse.tile as tile
from concourse import bacc
from concourse.bass_utils import run_bass_kernel_spmd

BF16 = mybir.dt.bfloat16
F32 = mybir.dt.float32
F32R = mybir.dt.float32r
AF = mybir.ActivationFunctionType

NCORES = 8
B, S, D = 2, 2048, 2048
H = 16
DN, DR, DV = 128, 64, 128
KVR, QR = 512, 1536
T = B * S            # 4096 flattened tokens
TS = T // NCORES     # 512 tokens per core (256 per batch)
HB = TS // B         # 256 tokens per batch per core
HL = H // NCORES     # 2 heads per core
EPS = 1.1920929e-7
BASE = 10000.0
SCALE = 1.0 / float(np.sqrt(DN + DR))

NT = 512             # token free-dim tile
P = 128
KD = D // P          # 16 contraction tiles over model dim
MQ = QR // P         # 12
MC = KVR // P        # 4
NTT = T // NT        # 8 global token n-tiles
CKV = KVR + DR       # 576
KO = H * DV // P     # 16 contraction tiles for Wo
SB = S // NT         # 4 qt tiles per batch
QN_ROWS = H * DN     # 2048 nope rows (16 m-tiles, order j*2+h)
QR_ROWS = H * DR     # 1024 rope rows (8 m-tiles, order j)

_CACHE = {}


# ---------------------------------------------------------------- host side

def _deint_perm():
    return np.concatenate([np.arange(0, DR, 2), np.arange(1, DR, 2)])


def _rope_tables():
    t = np.arange(S, dtype=np.float32)
    inv = 1.0 / (BASE ** (np.arange(0, DR, 2, dtype=np.float32) / DR))
    ang = np.outer(t, inv)
    emb = np.concatenate([ang, ang], axis=-1)          # (S, DR)
    return np.cos(emb), np.sin(emb)


def _shard_tokens(c):
    b0 = np.arange(HB * c, HB * (c + 1))
    return np.concatenate([b0, S + b0])


def host_prep(inputs):
    x = np.asarray(inputs["x"], np.float32).reshape(T, D)
    Wqa = np.asarray(inputs["Wqa"], np.float32)
    gqa = np.asarray(inputs["gqa"], np.float32)
    Wqb = np.asarray(inputs["Wqb"], np.float32)
    Wkva = np.asarray(inputs["Wkva"], np.float32)
    gkva = np.asarray(inputs["gkva"], np.float32)
    Wkvb = np.asarray(inputs["Wkvb"], np.float32)
    Wo = np.asarray(inputs["Wo"], np.float32)

    bf = lambda a: np.ascontiguousarray(a).astype(ml_dtypes.bfloat16)
    perm = _deint_perm()

    wqa_b = bf(Wqa)
    wkva_p = Wkva.copy()
    wkva_p[:, KVR:] = Wkva[:, KVR:][:, perm]
    wkva_b = bf(wkva_p)

    # full Wqb, columns regrouped into 8 head-pair blocks of 384:
    # [h(2j) nope | h(2j+1) nope | h(2j) rope perm | h(2j+1) rope perm]
    blocks = []
    for j in range(NCORES):
        b0 = Wqb[:, (2 * j) * (DN + DR):(2 * j + 1) * (DN + DR)]
        b1 = Wqb[:, (2 * j + 1) * (DN + DR):(2 * j + 2) * (DN + DR)]
        blocks += [b0[:, :DN], b1[:, :DN],
                   b0[:, DN:][:, perm], b1[:, DN:][:, perm]]
    wqb_full = bf(np.concatenate(blocks, axis=1)
                  * gqa[:, None] * SCALE)              # (QR, 3072)

    cos, sin = _rope_tables()                          # (S, 64)
    cosT, sinT = cos.T, sin.T                          # (64, S)

    ones_col = np.ones((P, 1), np.float32)
    ones_row = np.ones((1, P), np.float32)
    # rotate-half as a matmul: out = rotm.T @ in  (block-diag over 2 heads)
    HD = DR // 2
    rotm = np.zeros((P, P), np.float32)
    for bb in (0, DR):
        for dout in range(HD):
            rotm[bb + dout + HD, bb + dout] = -1.0
        for dout in range(HD, DR):
            rotm[bb + dout - HD, bb + dout] = 1.0
    rotm_b = bf(rotm)
    wo_b = bf(Wo)

    # diagonal-tile masks, flattened to (128, 4*NT): block jm at cols jm*NT
    qcol = np.arange(NT)
    rr = np.arange(P)
    masks = np.zeros((P, 4 * NT), np.float32)
    for jm in range(4):
        masks[:, jm * NT:(jm + 1) * NT] = (
            qcol[None, :] >= (128 * jm + rr)[:, None])
    masks_b = bf(masks)

    in_maps = []
    for c in range(NCORES):
        toks = _shard_tokens(c)
        xT = bf(x[toks].T)                             # (2048, 512)
        h0, h1 = HL * c, HL * c + 1

        kb0 = Wkvb[:, h0 * (DN + DV):(h0 + 1) * (DN + DV)]
        kb1 = Wkvb[:, h1 * (DN + DV):(h1 + 1) * (DN + DV)]
        wkvbk_c = bf(np.concatenate([kb0[:, :DN], kb1[:, :DN]], axis=1)
                     * gkva[:, None])
        wkvbv_c = bf(np.concatenate([kb0[:, DN:], kb1[:, DN:]], axis=1)
                     * gkva[:, None])

        pos = toks % S
        # cos/sin for this core's tokens, duplicated across both head
        # halves of a rope m-tile (rows 0-63 and 64-127)
        cs = np.ascontiguousarray(cosT[:, pos]).astype(np.float32)
        sn = np.ascontiguousarray(sinT[:, pos]).astype(np.float32)
        cos_s1 = np.concatenate([cs, cs], axis=0)      # (128, TS)
        sin_s1 = np.concatenate([sn, sn], axis=0)

        in_maps.append({
            "xT": xT, "wqa": wqa_b, "wkva": wkva_b, "wqb": wqb_full,
            "wkvbk": wkvbk_c, "wkvbv": wkvbv_c, "wo": wo_b,
            "cos_s1": cos_s1, "sin_s1": sin_s1,
            "masks": masks_b, "ones_col": ones_col, "ones_row": ones_row,
            "rotm": rotm_b,
        })
    return in_maps


# ---------------------------------------------------------------- device IR

def build_nc(do_compile=True):
    nc = bacc.Bacc(
        "TRN2", target_bir_lowering=False, debug=False,
        enable_asserts=True, num_devices=NCORES,
    )
    xT = nc.dram_tensor("xT", [D, TS], BF16, kind="ExternalInput")
    wqa = nc.dram_tensor("wqa", [D, QR], BF16, kind="ExternalInput")
    wkva = nc.dram_tensor("wkva", [D, CKV], BF16, kind="ExternalInput")
    wqb = nc.dram_tensor("wqb", [QR, H * (DN + DR)], BF16,
                         kind="ExternalInput")
    wkvbk = nc.dram_tensor("wkvbk", [KVR, HL * DN], BF16, kind="ExternalInput")
    wkvbv = nc.dram_tensor("wkvbv", [KVR, HL * DV], BF16, kind="ExternalInput")
    wo = nc.dram_tensor("wo", [H * DV, D], BF16, kind="ExternalInput")
    cos_s1 = nc.dram_tensor("cos_s1", [P, TS], F32, kind="ExternalInput")
    sin_s1 = nc.dram_tensor("sin_s1", [P, TS], F32, kind="ExternalInput")
    masks = nc.dram_tensor("masks", [P, 4 * NT], BF16, kind="ExternalInput")
    ones_col = nc.dram_tensor("ones_col", [P, 1], F32R, kind="ExternalInput")
    ones_row = nc.dram_tensor("ones_row", [1, P], F32R, kind="ExternalInput")
    rotm = nc.dram_tensor("rotm", [P, P], BF16, kind="ExternalInput")
    out = nc.dram_tensor("out", [TS, D], F32, kind="ExternalOutput")

    RG = [list(range(NCORES))]

    with tile.TileContext(nc) as tc:
        with (
            tc.tile_pool(name="const", bufs=1) as cpool,
            tc.tile_pool(name="dram", bufs=1, space="DRAM") as dram,
        ):
            onc = cpool.tile([P, 1], F32R, name="onc")
            onr = cpool.tile([1, P], F32R, name="onr")
            nc.sync.dma_start(onc[:], ones_col[:])
            nc.sync.dma_start(onr[:], ones_row[:])
            mask_sb = cpool.tile([P, 4 * NT], BF16, name="mask_sb")
            nc.sync.dma_start(mask_sb[:], masks[:])
            eps_sb = cpool.tile([1, 1], F32, name="eps_sb")
            nc.vector.memset(eps_sb[:], EPS)
            rotm_sb = cpool.tile([P, P], BF16, name="rotm_sb")
            nc.sync.dma_start(rotm_sb[:], rotm[:])

            cc_kv_in = dram.tile([CKV, TS], BF16, name="cc_kv_in")
            cc_kv_out = dram.tile([NCORES * CKV, TS], BF16,
                                  addr_space="Shared", name="cc_kv_out")
            cc_q_in = dram.tile([H * (DN + DR), TS], BF16, name="cc_q_in")
            cc_q_out = dram.tile([H * (DN + DR), TS], BF16, name="cc_q_out")
            cc_ao_in = [dram.tile([NCORES * HL * DV, HB], BF16,
                                  name=f"cc_ao_in{b}") for b in range(B)]
            cc_ao_out = [dram.tile([NCORES * HL * DV, HB], BF16,
                                   name=f"cc_ao_out{b}") for b in range(B)]

            # =================== stage 1 =================================
            with (
                tc.tile_pool(name="s1sb", bufs=1) as s1,
                tc.tile_pool(name="s1ps", bufs=1, space="PSUM") as ps1,
            ):
                xT_sb = s1.tile([P, KD * TS], BF16, name="xT_sb")
                wkva_sb = s1.tile([P, KD * CKV], BF16, name="wkva_sb")
                wqa_sb = s1.tile([P, KD * QR], BF16, name="wqa_sb")
                # xT + wkva first so the kv-side matmuls (which feed the
                # AllGather) are never stuck behind the 6MB wqa load
                for k in range(KD):
                    nc.sync.dma_start(xT_sb[:, k * TS:(k + 1) * TS],
                                      xT[k * P:(k + 1) * P, :])
                    nc.sync.dma_start(wkva_sb[:, k * CKV:(k + 1) * CKV],
                                      wkva[k * P:(k + 1) * P, :])
                for k in range(KD):
                    nc.sync.dma_start(wqa_sb[:, k * QR:(k + 1) * QR],
                                      wqa[k * P:(k + 1) * P, :])
                cos1_sb = s1.tile([P, TS], F32, name="cos1_sb")
                sin1_sb = s1.tile([P, TS], F32, name="sin1_sb")
                nc.sync.dma_start(cos1_sb[:], cos_s1[:])
                nc.sync.dma_start(sin1_sb[:], sin_s1[:])

                # ---- kv side first (its AllGather overlaps the q side)
                c_ps = []
                for m in range(MC):
                    ps = ps1.tile([P, TS], F32, tag="s1mm", bufs=4,
                                  name=f"ckv_ps{m}")
                    for k in range(KD):
                        nc.tensor.matmul(
                            ps[:],
                            wkva_sb[:, k * CKV + m * P:k * CKV + (m + 1) * P],
                            xT_sb[:, k * TS:(k + 1) * TS],
                            start=(k == 0), stop=(k == KD - 1))
                    c_ps.append(ps)
                kr_ps = ps1.tile([DR, TS], F32, tag="s1kr", bufs=2,
                                 name="kr_ps")
                for k in range(KD):
                    nc.tensor.matmul(
                        kr_ps[:],
                        wkva_sb[:, k * CKV + KVR:k * CKV + KVR + DR],
                        xT_sb[:, k * TS:(k + 1) * TS],
                        start=(k == 0), stop=(k == KD - 1))

                # rotate-half via PE: krot = rotm.T @ kr
                kraw = s1.tile([DR, TS], BF16, name="kraw")
                nc.scalar.copy(kraw[:], kr_ps[:])
                krot_ps = ps1.tile([DR, TS], F32, tag="s1kr", bufs=2,
                                   name="krot_ps")
                nc.tensor.matmul(krot_ps[:], rotm_sb[0:DR, 0:DR], kraw[:],
                                 start=True, stop=True)
                t1 = s1.tile([DR, TS], F32, name="t1")
                t2 = s1.tile([DR, TS], F32, name="t2")
                nc.vector.tensor_mul(t1[:], kr_ps[:], cos1_sb[0:DR, :])
                nc.vector.tensor_mul(t2[:], krot_ps[:], sin1_sb[0:DR, :])
                kro_sb = s1.tile([DR, TS], BF16, name="kro_sb")
                nc.vector.tensor_add(kro_sb[:], t1[:], t2[:])
                nc.sync.dma_start(cc_kv_in[KVR:CKV, :], kro_sb[:])

                ss_ps = ps1.tile([1, TS], F32, tag="s1row", bufs=1,
                                 name="ss_kv")
                for m in range(MC):
                    sq = s1.tile([P, TS], F32R, tag="sq", bufs=2,
                                 name=f"sqkv{m}")
                    nc.scalar.square(sq[:], c_ps[m][:])
                    nc.tensor.matmul(ss_ps[:], onc[:], sq[:],
                                     start=(m == 0), stop=(m == MC - 1))
                srow = s1.tile([1, TS], F32, tag="srow", bufs=2,
                               name="srow_kv")
                nc.scalar.activation(srow[:], ss_ps[:], AF.Sqrt,
                                     bias=eps_sb[:], scale=1.0 / KVR)
                rrow = s1.tile([1, TS], F32R, tag="rrow", bufs=2,
                               name="rrow_kv")
                with nc.allow_low_precision(reason="f32r feeds f32r matmul"):
                    nc.vector.reciprocal(rrow[:], srow[:])
                bc_ps = ps1.tile([P, TS], F32, tag="s1bc", bufs=1,
                                 name="bc_kv")
                nc.tensor.matmul(bc_ps[:], onr[:], rrow[:],
                                 start=True, stop=True)
                bc_sb = s1.tile([P, TS], F32, tag="bcs", bufs=2,
                                name="bcs_kv")
                nc.scalar.copy(bc_sb[:], bc_ps[:])
                for m in range(MC):
                    cn = s1.tile([P, TS], BF16, tag="cn", bufs=2,
                                 name=f"cn{m}")
                    nc.vector.tensor_mul(cn[:], c_ps[m][:], bc_sb[:])
                    nc.sync.dma_start(cc_kv_in[m * P:(m + 1) * P, :], cn[:])

                nc.gpsimd.collective_compute(
                    "AllGather", mybir.AluOpType.bypass, replica_groups=RG,
                    ins=[cc_kv_in.opt()], outs=[cc_kv_out.opt()])

                # ---- q side: q_a = x @ Wqa (raw, bf16); the rms scale is
                # applied to the Wqb projection outputs instead (the
                # per-token scale commutes with the feature contraction).
                ssq_ps = ps1.tile([1, TS], F32, tag="s1row", bufs=1,
                                  name="ss_q")
                qa_raw = []
                for m in range(MQ):
                    ps = ps1.tile([P, TS], F32, tag="s1mm", bufs=4,
                                  name=f"qa_ps{m}")
                    for k in range(KD):
                        nc.tensor.matmul(
                            ps[:],
                            wqa_sb[:, k * QR + m * P:k * QR + (m + 1) * P],
                            xT_sb[:, k * TS:(k + 1) * TS],
                            start=(k == 0), stop=(k == KD - 1))
                    raw = s1.tile([P, TS], BF16, tag=f"qraw{m}", bufs=1,
                                  name=f"qraw{m}")
                    nc.vector.tensor_copy(raw[:], ps[:])
                    qa_raw.append(raw)
                    sq = s1.tile([P, TS], F32R, tag="sq", bufs=2,
                                 name=f"sqq{m}")
                    nc.scalar.square(sq[:], ps[:])
                    nc.tensor.matmul(ssq_ps[:], onc[:], sq[:],
                                     start=(m == 0), stop=(m == MQ - 1))
                srow_q = s1.tile([1, TS], F32, tag="srow", bufs=2,
                                 name="srow_q")
                nc.scalar.activation(srow_q[:], ssq_ps[:], AF.Sqrt,
                                     bias=eps_sb[:], scale=1.0 / QR)
                rrow_q = s1.tile([1, TS], F32R, tag="rrow", bufs=2,
                                 name="rrow_q")
                with nc.allow_low_precision(reason="f32r feeds f32r matmul"):
                    nc.vector.reciprocal(rrow_q[:], srow_q[:])
                bcq_ps = ps1.tile([P, TS], F32, tag="s1bc", bufs=1,
                                  name="bc_q")
                nc.tensor.matmul(bcq_ps[:], onr[:], rrow_q[:],
                                 start=True, stop=True)
                bcq_sb = s1.tile([P, TS], F32, tag="bcs", bufs=2,
                                 name="bcs_q")
                nc.scalar.copy(bcq_sb[:], bcq_ps[:])

                # ---- full q projection (8 head-pair blocks of 3 m-tiles:
                # nope h0, nope h1, rope pair), wqb streamed per block
                HPB = 3 * P       # 384 columns per head-pair block
                for j in range(NCORES):
                    wq = s1.tile([P, MQ * HPB], BF16, tag="wqn", bufs=2,
                                 name=f"wqn{j}")
                    for k in range(MQ):
                        nc.sync.dma_start(
                            wq[:, k * HPB:(k + 1) * HPB],
                            wqb[k * P:(k + 1) * P, j * HPB:(j + 1) * HPB])
                    for m in range(3):
                        ps = ps1.tile([P, TS], F32, tag="s1mm", bufs=4,
                                      name="qf_ps")
                        for k in range(MQ):
                            nc.tensor.matmul(
                                ps[:],
                                wq[:, k * HPB + m * P:
                                   k * HPB + (m + 1) * P],
                                qa_raw[k][:],
                                start=(k == 0), stop=(k == MQ - 1))
                        if m < 2:
                            qf = s1.tile([P, TS], BF16, tag="qf", bufs=3,
                                         name="qf")
                            nc.vector.tensor_mul(qf[:], ps[:], bcq_sb[:])
                            nc.sync.dma_start(
                                cc_q_in[j * HPB + m * P:
                                        j * HPB + (m + 1) * P, :], qf[:])
                        else:
                            qrw = s1.tile([P, TS], BF16, tag="qrw", bufs=2,
                                          name="qrw")
                            nc.scalar.copy(qrw[:], ps[:])
                            rps = ps1.tile([P, TS], F32, tag="s1kr", bufs=2,
                                           name="rps")
                            nc.tensor.matmul(rps[:], rotm_sb[:], qrw[:],
                                             start=True, stop=True)
                            u1 = s1.tile([P, TS], F32, tag="u1", bufs=2,
                                         name="u1")
                            u2 = s1.tile([P, TS], F32, tag="u2", bufs=2,
                                         name="u2")
                            nc.vector.tensor_mul(u1[:], ps[:], cos1_sb[:])
                            nc.vector.tensor_mul(u2[:], rps[:], sin1_sb[:])
                            u3 = s1.tile([P, TS], F32, tag="u3", bufs=2,
                                         name="u3")
                            nc.vector.tensor_add(u3[:], u1[:], u2[:])
                            qro = s1.tile([P, TS], BF16, tag="qf", bufs=3,
                                          name="qro")
                            nc.vector.tensor_mul(qro[:], u3[:], bcq_sb[:])
                            nc.sync.dma_start(
                                cc_q_in[j * HPB + 2 * P:
                                        j * HPB + 3 * P, :], qro[:])

                nc.gpsimd.collective_compute(
                    "AllToAll", mybir.AluOpType.bypass, replica_groups=RG,
                    ins=[cc_q_in.opt()], outs=[cc_q_out.opt()])

            # =================== stage 2 =================================
            ckv_g = cc_kv_out.rearrange("(r p) t -> r p t", r=NCORES)
            q_g = cc_q_out.rearrange("(r f) t -> r f t", r=NCORES)

            with (
                tc.tile_pool(name="s2sb", bufs=1) as s2,
                tc.tile_pool(name="attnsb", bufs=1) as sA,
            ):
                kn_sb = s2.tile([P, HL * T], BF16, name="kn_sb")
                qn_sb = s2.tile([P, HL * T], BF16, name="qn_sb")
                qr_sb = s2.tile([P, T], BF16, name="qr_sb")
                # shared rotated rope key, duplicated in both partition
                # halves so lhsT/rhs base partitions match per head
                kro2_sb = s2.tile([P, T], BF16, name="kro2_sb")
                v_tiles = [s2.tile([P, HL * DV], BF16, tag=f"v{tt}", bufs=1,
                                   name=f"v{tt}") for tt in range(T // P)]

                pj = tc.alloc_tile_pool(name="projsb", bufs=1)
                ps2a = tc.alloc_tile_pool(name="projps", bufs=2, space="PSUM")

                wkvbk_sb = pj.tile([P, MC * HL * DN], BF16, name="wkvbk_sb")
                wkvbv_sb = pj.tile([P, MC * HL * DV], BF16, name="wkvbv_sb")
                for k in range(MC):
                    nc.sync.dma_start(
                        wkvbk_sb[:, k * HL * DN:(k + 1) * HL * DN],
                        wkvbk[k * P:(k + 1) * P, :])
                    nc.sync.dma_start(
                        wkvbv_sb[:, k * HL * DV:(k + 1) * HL * DV],
                        wkvbv[k * P:(k + 1) * P, :])

                # assemble gathered c_kv / rope key into natural order
                c_sb = pj.tile([P, MC * T], BF16, name="c_sb")
                c_v = c_sb.rearrange("p (k b t) -> p k b t", k=MC, b=B)
                kro_v0 = kro2_sb[0:DR, :].rearrange("p (b t) -> p b t", b=B)
                kro_v1 = kro2_sb[DR:P, :].rearrange("p (b t) -> p b t", b=B)
                for r in range(NCORES):
                    src = ckv_g[r].rearrange("p (b t) -> p b t", b=B)
                    for k in range(MC):
                        nc.sync.dma_start(
                            c_v[:, k, :, HB * r:HB * (r + 1)],
                            src[k * P:(k + 1) * P, :, :])
                    nc.sync.dma_start(
                        kro_v0[:, :, HB * r:HB * (r + 1)],
                        src[KVR:CKV, :, :])
                    nc.sync.dma_start(
                        kro_v1[:, :, HB * r:HB * (r + 1)],
                        src[KVR:CKV, :, :])

                # kT projection
                for h in range(HL):
                    for n in range(NTT):
                        ps = ps2a.tile([P, NT], F32, tag="proj",
                                       name="kn_ps")
                        for k in range(MC):
                            nc.tensor.matmul(
                                ps[:],
                                wkvbk_sb[:, k * HL * DN + h * DN:
                                         k * HL * DN + (h + 1) * DN],
                                c_sb[:, k * T + n * NT:
                                     k * T + (n + 1) * NT],
                                start=(k == 0), stop=(k == MC - 1))
                        nc.scalar.copy(
                            kn_sb[:, h * T + n * NT:h * T + (n + 1) * NT],
                            ps[:])

                # v projection (natural layout)
                for tt in range(T // P):
                    ps = ps2a.tile([P, HL * DV], F32, tag="proj",
                                   name="v_ps")
                    for k in range(MC):
                        nc.tensor.matmul(
                            ps[:],
                            c_sb[:, k * T + tt * P:k * T + (tt + 1) * P],
                            wkvbv_sb[:, k * HL * DV:(k + 1) * HL * DV],
                            start=(k == 0), stop=(k == MC - 1))
                    nc.scalar.copy(v_tiles[tt][:], ps[:])

                # assemble this core's 2 heads' q from the A2A outputs
                qn_v = [qn_sb[:, h * T:(h + 1) * T]
                        .rearrange("p (b t) -> p b t", b=B)
                        for h in range(HL)]
                qr_v = qr_sb.rearrange("p (b t) -> p b t", b=B)
                for r in range(NCORES):
                    src = q_g[r].rearrange("f (b t) -> f b t", b=B)
                    for h in range(HL):
                        nc.sync.dma_start(
                            qn_v[h][:, :, HB * r:HB * (r + 1)],
                            src[h * P:(h + 1) * P, :, :])
                    nc.sync.dma_start(
                        qr_v[:, :, HB * r:HB * (r + 1)],
                        src[2 * P:3 * P, :, :])

                pj.release()
                ps2a.release()

                # ---- Wo weights: load early, overlaps attention ----
                wp = tc.alloc_tile_pool(name="wosb", bufs=1)
                wo_sb = wp.tile([P, KO * D], BF16, name="wo_sb")
                for k in range(KO):
                    nc.sync.dma_start(wo_sb[:, k * D:(k + 1) * D],
                                      wo[k * P:(k + 1) * P, :])

                # ---- attention ----
                psA = tc.alloc_tile_pool(name="attnps", bufs=1, space="PSUM")
                ao_sb = [sA.tile([DV, T], BF16, name=f"ao_sb{h}")
                         for h in range(HL)]
                last_ao_dma = [None, None]
                for b in range(B):
                    for h in range(HL):
                        for qti in range(SB):
                            qs = b * S + qti * NT
                            aop = psA.tile([DV, NT], F32, tag="ao", bufs=2,
                                           name="aop")
                            dn = sA.tile([P, NT], F32R, tag="dn", bufs=2,
                                         name="dn")
                            nk = 4 * qti + 4
                            for kp in range(nk // 2):
                                scp = psA.tile([P, 2 * NT], F32, tag="sc",
                                               bufs=2, name="scp")
                                for half in range(2):
                                    kti = 2 * kp + half
                                    ks = b * S + kti * P
                                    sl = scp[:, half * NT:(half + 1) * NT]
                                    nc.tensor.matmul(
                                        sl,
                                        kn_sb[:, h * T + ks:h * T + ks + P],
                                        qn_sb[:, h * T + qs:
                                              h * T + qs + NT],
                                        start=True, stop=False)
                                    nc.tensor.matmul(
                                        sl,
                                        kro2_sb[h * DR:(h + 1) * DR,
                                                ks:ks + P],
                                        qr_sb[h * DR:(h + 1) * DR,
                                              qs:qs + NT],
                                        start=False, stop=True)
                                et = sA.tile([P, 2 * NT], BF16, tag="et",
                                             bufs=3, name="et")
                                nc.scalar.activation(et[:], scp[:], AF.Exp)
                                for half in range(2):
                                    kti = 2 * kp + half
                                    eth = et[:, half * NT:(half + 1) * NT]
                                    if kti >= 4 * qti:
                                        jm = kti % 4
                                        nc.vector.tensor_mul(
                                            eth, eth,
                                            mask_sb[:,
                                                    jm * NT:(jm + 1) * NT])
                                    if kti == 0:
                                        nc.vector.tensor_copy(dn[:], eth)
                                    else:
                                        nc.vector.tensor_add(dn[:], dn[:],
                                                             eth)
                                    tt = (b * S) // P + kti
                                    nc.tensor.matmul(
                                        aop[:],
                                        v_tiles[tt][:, h * DV:(h + 1) * DV],
                                        eth,
                                        start=(kti == 0),
                                        stop=(kti == nk - 1))
                            dps = psA.tile([1, NT], F32, tag="drow", bufs=1,
                                           name="dps")
                            nc.tensor.matmul(dps[:], onc[:], dn[:],
                                             start=True, stop=True)
                            rec = sA.tile([1, NT], F32R, tag="rec", bufs=2,
                                          name="rec")
                            with nc.allow_low_precision(
                                    reason="f32r feeds f32r matmul"):
                                nc.vector.reciprocal(rec[:], dps[:])
                            bcp = psA.tile([P, NT], F32, tag="bc", bufs=1,
                                           name="bcp")
                            nc.tensor.matmul(bcp[:], onr[:], rec[:],
                                             start=True, stop=True)
                            bcs = sA.tile([P, NT], F32, tag="bcs2", bufs=2,
                                          name="bcs")
                            nc.vector.tensor_copy(bcs[:], bcp[:])
                            nc.vector.tensor_mul(
                                ao_sb[h][:, qs:qs + NT], aop[:], bcs[:])
                    # AllToAll this batch: shard j = (my heads, core j toks)
                    for j in range(NCORES):
                        for h in range(HL):
                            last_ao_dma[b] = nc.sync.dma_start(
                                cc_ao_in[b][(j * HL + h) * DV:
                                            (j * HL + h + 1) * DV, :],
                                ao_sb[h][:, b * S + HB * j:
                                         b * S + HB * (j + 1)])
                    nc.gpsimd.collective_compute(
                        "AllToAll", mybir.AluOpType.bypass, replica_groups=RG,
                        ins=[cc_ao_in[b].opt()], outs=[cc_ao_out[b].opt()])
                psA.release()

                # ---- Wo (full contraction over all 16 heads) ----
                ps2b = tc.alloc_tile_pool(name="wops", bufs=2, space="PSUM")

                def wo_block(b):
                    first = True
                    for mb in range(HB // P):            # 2 m-tiles per batch
                        aog = []
                        for k in range(KO):
                            ag = wp.tile([P, P], BF16, tag="aog",
                                         bufs=KO + 2, name=f"aog{k}")
                            d = nc.sync.dma_start(
                                ag[:],
                                cc_ao_out[b][k * P:(k + 1) * P,
                                             mb * P:(mb + 1) * P])
                            if first:
                                # keep this load behind batch 1's A2A input
                                # stores in the SP HWDGE FIFO (head-of-line)
                                tile.add_dep_helper(
                                    d.ins, last_ao_dma[B - 1].ins,
                                    sync=False,
                                    reason="wo aog after last a2a input")
                                first = False
                            aog.append(ag)
                        for n in range(D // NT):
                            ps = ps2b.tile([P, NT], F32, tag="wo",
                                           name="wo_ps")
                            for k in range(KO):
                                nc.tensor.matmul(
                                    ps[:], aog[k][:],
                                    wo_sb[:, k * D + n * NT:
                                          k * D + (n + 1) * NT],
                                    start=(k == 0), stop=(k == KO - 1))
                            ob = wp.tile([P, NT], F32, tag="ob", bufs=3,
                                         name="ob")
                            nc.scalar.copy(ob[:], ps[:])
                            nc.sync.dma_start(
                                out[(b * 2 + mb) * P:
                                    (b * 2 + mb + 1) * P,
                                    n * NT:(n + 1) * NT], ob[:])

                wo_block(0)
                wo_block(1)
                ps2b.release()
                wp.release()

    if do_compile:
        nc.compile()
    return nc


# ---------------------------------------------------------------- entry

def _get_nc():
    if "nc" not in _CACHE:
        _CACHE["nc"] = build_nc()
    return _CACHE["nc"]


def kernel(**inputs):
    nc = _get_nc()
    in_maps = host_prep(inputs)
    res = run_bass_kernel_spmd(nc, in_maps, core_ids=list(range(NCORES)))
    outg = np.zeros((T, D), np.float32)
    for c in range(NCORES):
        outg[_shard_tokens(c)] = res.results[c]["out"]
    return outg.reshape(B, S, D)
